# revision 5
# baseline (speedup 1.0000x reference)
"""Distributed Trainium2 kernel for GQA causal attention (B=2, L=2048, DIM=2048,
H=32 q-heads, KV=8 kv-heads, HD=64), tensor-parallel over heads across 8 cores.

Per-core pipeline (core r owns q heads 4r..4r+3 and kv head r):
  P1: qkv = x @ [wqT|wkT|wvT] shard (bf16 matmuls, f32 psum), RoPE on q/k,
      PE-transpose q/k into [head_dim, seq] layout; keep v in [seq, head_dim].
  P2: causal attention per (batch, head) with scores computed transposed
      (S^T[j,i] so softmax probs feed the PV matmul without transposing P),
      exp without max-subtraction (scores are O(5), f32 psum), denominator
      fused as a ones-column of the V stationary, per-512-column chunks.
  P2.5: AllToAll switches head-sharding -> sequence-sharding (2MB bf16).
  P3: full output projection for this core's 512-row sequence shard.
Host: concatenates the 8 row-shards.
"""

import sys

sys.path.insert(0, "/opt/trn_rl_repo")

import numpy as np
import ml_dtypes

from concourse import bass, bacc, mybir, tile
from concourse.bass_utils import run_bass_kernel_spmd

BF16 = ml_dtypes.bfloat16
FP32 = np.float32

R = 8            # cores
B, L, DIM = 2, 2048, 2048
H, KV, HD = 32, 8, 64
HL = H // R      # 4 local q heads per core
BL = B * L       # 4096
NT = BL // 128   # 32 row tiles
KT = DIM // 128  # 16 contraction tiles
CL = HL * HD     # 256 local q/out dims
NCH = L // 512   # 4 i-chunks per batch
SH = BL // R     # 512 seq rows per core in phase 3

_BF = mybir.dt.bfloat16
_F32 = mybir.dt.float32

_cache = {}


def _emit(nc, t):
    mult = mybir.AluOpType.mult
    add = mybir.AluOpType.add
    sub = mybir.AluOpType.subtract
    Exp = mybir.ActivationFunctionType.Exp
    Copy = mybir.ActivationFunctionType.Copy
    tc = t["tc"]

    with tc.tile_pool(name="persist", bufs=1) as P, \
         tc.tile_pool(name="dramp", bufs=1, space="DRAM") as DP:
        wcomb_sb = P.tile([128, KT * 384], _BF, name="wcomb_sb")
        cosq_sb = P.tile([128, NT * 32], _F32, name="cosq_sb")
        sinq_sb = P.tile([128, NT * 32], _F32, name="sinq_sb")
        cosk_sb = P.tile([128, NT * 32], _F32, name="cosk_sb")
        sink_sb = P.tile([128, NT * 32], _F32, name="sink_sb")
        mask_sb = P.tile([128, 128], _BF, name="mask_sb")
        ident_sb = P.tile([128, 128], _BF, name="ident_sb")
        qT = P.tile([64, HL * BL], _BF, name="qT")
        kT = P.tile([64, BL], _BF, name="kT")
        v1 = P.tile([128, NT * 65], _BF, name="v1")
        out_sb = P.tile([128, NT * CL], _BF, name="out_sb")
        wot_sb = P.tile([128, KT * DIM], _BF, name="wot_sb")

        a2a_in = DP.tile([BL, CL], _BF, name="a2a_in")
        a2a_out = DP.tile([BL, CL], _BF, name="a2a_out")

        # --- constant / weight loads -------------------------------------
        nc.sync.dma_start(
            out=wcomb_sb[:].rearrange("p (k c) -> p k c", k=KT),
            in_=t["wcomb"].ap().rearrange("k p c -> p k c"),
        )
        for name, sb in (("cosq", cosq_sb), ("sinq", sinq_sb),
                         ("cosk", cosk_sb), ("sink", sink_sb)):
            nc.sync.dma_start(
                out=sb[:].rearrange("p (t c) -> p t c", t=NT),
                in_=t[name].ap().rearrange("t p c -> p t c"),
            )
        nc.sync.dma_start(out=mask_sb[:], in_=t["mask"].ap())
        nc.sync.dma_start(out=ident_sb[:], in_=t["ident"].ap())
        nc.vector.memset(
            v1[:].rearrange("p (t c) -> p t c", c=65)[:, :, 64:65], 1.0
        )

        # --- phase 1: qkv projection + rope + transposes ------------------
        with tc.tile_pool(name="pp1", bufs=1, space="PSUM") as pp1, \
             tc.tile_pool(name="sp1", bufs=1) as sp1:
            for it in range(NT):
                xt_sb = sp1.tile([128, DIM], _BF, tag="xt", bufs=3, name=f"xt{it}")
                nc.sync.dma_start(out=xt_sb[:], in_=t["xt"].ap()[it])
                qkv = pp1.tile([128, 384], _F32, tag="qkv", bufs=2, name=f"qkv{it}")
                for kt in range(KT):
                    nc.tensor.matmul(
                        qkv[:],
                        xt_sb[:, kt * 128:(kt + 1) * 128],
                        wcomb_sb[:, kt * 384:(kt + 1) * 384],
                        start=(kt == 0), stop=(kt == KT - 1),
                    )
                # rope on q (4 heads, pairs interleaved along free dim)
                qrot = sp1.tile([128, CL], _BF, tag="qrot", bufs=2, name=f"qrot{it}")
                q3 = qkv[:, 0:CL].rearrange("p (h c) -> p h c", h=HL)
                qe, qo = q3[:, :, 0:64:2], q3[:, :, 1:64:2]
                cq = cosq_sb[:, it * 32:(it + 1) * 32].unsqueeze(1).broadcast_to([128, HL, 32])
                sq = sinq_sb[:, it * 32:(it + 1) * 32].unsqueeze(1).broadcast_to([128, HL, 32])
                t1 = sp1.tile([128, 128], _F32, tag="t1", bufs=2, name=f"t1_{it}")
                t2 = sp1.tile([128, 128], _F32, tag="t2", bufs=2, name=f"t2_{it}")
                t3 = sp1.tile([128, 128], _F32, tag="t3", bufs=2, name=f"t3_{it}")
                t4 = sp1.tile([128, 128], _F32, tag="t4", bufs=2, name=f"t4_{it}")
                v1_ = lambda x: x[:].rearrange("p (h c) -> p h c", h=HL)
                qr3 = qrot[:].rearrange("p (h c) -> p h c", h=HL)
                nc.vector.tensor_tensor(v1_(t1), qe, cq, mult)
                nc.vector.tensor_tensor(v1_(t2), qo, sq, mult)
                nc.vector.tensor_tensor(v1_(t3), qe, sq, mult)
                nc.vector.tensor_tensor(v1_(t4), qo, cq, mult)
                nc.vector.tensor_tensor(qr3[:, :, 0:64:2], v1_(t1), v1_(t2), sub)
                nc.vector.tensor_tensor(qr3[:, :, 1:64:2], v1_(t3), v1_(t4), add)
                # rope on k (single kv head)
                krot = sp1.tile([128, 64], _BF, tag="krot", bufs=2, name=f"krot{it}")
                kbase = qkv[:, CL:CL + 64]
                ke, ko = kbase[:, 0:64:2], kbase[:, 1:64:2]
                ck = cosk_sb[:, it * 32:(it + 1) * 32]
                sk = sink_sb[:, it * 32:(it + 1) * 32]
                k1 = sp1.tile([128, 32], _F32, tag="k1", bufs=2, name=f"k1_{it}")
                k2 = sp1.tile([128, 32], _F32, tag="k2", bufs=2, name=f"k2_{it}")
                k3 = sp1.tile([128, 32], _F32, tag="k3", bufs=2, name=f"k3_{it}")
                k4 = sp1.tile([128, 32], _F32, tag="k4", bufs=2, name=f"k4_{it}")
                nc.vector.tensor_tensor(k1[:], ke, ck, mult)
                nc.vector.tensor_tensor(k2[:], ko, sk, mult)
                nc.vector.tensor_tensor(k3[:], ke, sk, mult)
                nc.vector.tensor_tensor(k4[:], ko, ck, mult)
                nc.vector.tensor_tensor(krot[:, 0:64:2], k1[:], k2[:], sub)
                nc.vector.tensor_tensor(krot[:, 1:64:2], k3[:], k4[:], add)
                # v copy (f32 psum -> bf16 sbuf)
                nc.scalar.activation(v1[:, it * 65:it * 65 + 64], qkv[:, 320:384], Copy)
                # transposes: q -> qT, k -> kT
                for h in range(HL):
                    trq = pp1.tile([64, 128], _BF, tag="trq", bufs=2,
                                   name=f"trq{it}_{h}")
                    nc.tensor.transpose(trq[:], qrot[:, h * 64:(h + 1) * 64],
                                        ident_sb[:])
                    nc.scalar.activation(
                        qT[:, h * BL + it * 128: h * BL + (it + 1) * 128],
                        trq[:], Copy)
                trk = pp1.tile([64, 128], _BF, tag="trk", bufs=2, name=f"trk{it}")
                nc.tensor.transpose(trk[:], krot[:], ident_sb[:])
                nc.scalar.activation(kT[:, it * 128:(it + 1) * 128], trk[:], Copy)

        # wot loads (needed in phase 3; emitted here so DMA happens in the
        # background during phase 2)
        for ct in range(KT):
            nc.sync.dma_start(
                out=wot_sb[:, ct * DIM:(ct + 1) * DIM], in_=t["wot"].ap()[ct]
            )

        # --- phase 2: causal attention, scores transposed -----------------
        with tc.tile_pool(name="ppS", bufs=1, space="PSUM") as ppS, \
             tc.tile_pool(name="ppO", bufs=1, space="PSUM") as ppO, \
             tc.tile_pool(name="ppT", bufs=1, space="PSUM") as ppT, \
             tc.tile_pool(name="sp2", bufs=1) as sp2:
            for b in range(B):
                for c in range(NCH):
                    i0 = c * 512
                    o_ps = [
                        ppO.tile([65, 512], _F32, tag=f"o{h}", name=f"o{h}_{b}_{c}")
                        for h in range(HL)
                    ]
                    njt = 4 * c + 4
                    for jt in range(njt):
                        j0 = jt * 128
                        a = max(0, j0 - i0)
                        n = 512 - a
                        for h in range(HL):
                            s_ps = ppS.tile([128, 512], _F32, tag="S", bufs=2,
                                            name=f"s{b}_{c}_{jt}_{h}")
                            qcol = h * BL + b * L + i0 + a
                            nc.tensor.matmul(
                                s_ps[:, a:512],
                                kT[0:64, b * L + j0: b * L + j0 + 128],
                                qT[0:64, qcol:qcol + n],
                                start=True, stop=True,
                            )
                            p_sb = sp2.tile([128, 512], _BF, tag="P", bufs=6,
                                            name=f"p{b}_{c}_{jt}_{h}")
                            nc.scalar.activation(p_sb[:, a:512], s_ps[:, a:512], Exp)
                            if jt >= 4 * c:
                                nc.vector.tensor_tensor(
                                    p_sb[:, a:a + 128], p_sb[:, a:a + 128],
                                    mask_sb[:], mult)
                            nc.tensor.matmul(
                                o_ps[h][0:65, a:512],
                                v1[:, (b * 16 + jt) * 65:(b * 16 + jt) * 65 + 65],
                                p_sb[:, a:512],
                                start=(jt == 0), stop=(jt == njt - 1),
                                skip_group_check=True,
                            )
                    # normalize + transpose to [seq, head_dim]
                    for h in range(HL):
                        o_sb = sp2.tile([65, 512], _BF, tag="osb", bufs=2,
                                        name=f"osb{b}_{c}_{h}")
                        nc.vector.tensor_copy(o_sb[:], o_ps[h][0:65, :])
                        for q4 in range(4):
                            ot_ps = ppT.tile([128, 65], _BF, tag="ot", bufs=2,
                                             name=f"ot{b}_{c}_{h}_{q4}")
                            nc.tensor.transpose(
                                ot_ps[:], o_sb[:, q4 * 128:(q4 + 1) * 128],
                                ident_sb[0:65, 0:65])
                            rcp = sp2.tile([128, 1], _F32, tag="rcp", bufs=2,
                                           name=f"rcp{b}_{c}_{h}_{q4}")
                            nc.vector.reciprocal(rcp[:], ot_ps[:, 64:65])
                            git = b * 16 + c * 4 + q4
                            nc.vector.tensor_scalar_mul(
                                out_sb[:, git * CL + h * 64: git * CL + h * 64 + 64],
                                ot_ps[:, 0:64], rcp[:])
                    # stage this chunk's rows for the all-to-all
                    for q4 in range(4):
                        git = b * 16 + c * 4 + q4
                        nc.sync.dma_start(
                            out=a2a_in[git * 128:(git + 1) * 128, :],
                            in_=out_sb[:, git * CL:(git + 1) * CL])

        # --- phase 2.5: all-to-all (head shard -> seq shard) ---------------
        nc.gpsimd.collective_compute(
            "AllToAll", mybir.AluOpType.bypass,
            replica_groups=[list(range(R))],
            ins=[a2a_in[:].opt()],
            outs=[a2a_out[:].opt()],
        )

        # --- phase 3: output projection for this core's 512 seq rows ------
        with tc.tile_pool(name="pp3", bufs=1, space="PSUM") as pp3, \
             tc.tile_pool(name="sp3", bufs=1) as sp3:
            lhs = []
            for ct in range(KT):
                lt = sp3.tile([128, 512], _BF, tag=f"lhs{ct}", name=f"lhs{ct}")
                s, ch = ct // 2, ct % 2
                nc.sync.dma_start(
                    out=lt[:],
                    in_=a2a_out[512 * s:512 * (s + 1), ch * 128:(ch + 1) * 128],
                    transpose=True)
                lhs.append(lt)
            for it in range(SH // 128):
                for nck in range(4):
                    y_ps = pp3.tile([128, 512], _F32, tag="y", bufs=2,
                                    name=f"y{it}_{nck}")
                    for ct in range(KT):
                        nc.tensor.matmul(
                            y_ps[:],
                            lhs[ct][:, it * 128:(it + 1) * 128],
                            wot_sb[:, ct * DIM + nck * 512: ct * DIM + nck * 512 + 512],
                            start=(ct == 0), stop=(ct == KT - 1))
                    y_sb = sp3.tile([128, 512], _F32, tag="ysb", bufs=3,
                                    name=f"ysb{it}_{nck}")
                    nc.scalar.activation(y_sb[:], y_ps[:], Copy)
                    nc.sync.dma_start(
                        out=t["out"][it * 128:(it + 1) * 128,
                                     nck * 512:(nck + 1) * 512],
                        in_=y_sb[:])


def _build():
    if "nc" in _cache:
        return _cache["nc"]
    nc = bacc.Bacc("TRN2", target_bir_lowering=False, debug=False,
                   enable_asserts=False, num_devices=R)
    t = {}
    t["xt"] = nc.dram_tensor("xt", [NT, 128, DIM], _BF, kind="ExternalInput")
    t["wcomb"] = nc.dram_tensor("wcomb", [KT, 128, 384], _BF, kind="ExternalInput")
    for name in ("cosq", "sinq", "cosk", "sink"):
        t[name] = nc.dram_tensor(name, [NT, 128, 32], _F32, kind="ExternalInput")
    t["mask"] = nc.dram_tensor("mask", [128, 128], _BF, kind="ExternalInput")
    t["ident"] = nc.dram_tensor("ident", [128, 128], _BF, kind="ExternalInput")
    t["wot"] = nc.dram_tensor("wot", [KT, 128, DIM], _BF, kind="ExternalInput")
    t["out"] = nc.dram_tensor("out", [SH, DIM], _F32, kind="ExternalOutput")

    with tile.TileContext(nc) as tc:
        t["tc"] = tc
        _emit(nc, t)
    nc.compile()
    _cache["nc"] = nc
    return nc


def _prep_inputs(x, freqs_cis, wq, wk, wv, wo):
    x = np.asarray(x, dtype=FP32)
    freqs_cis = np.asarray(freqs_cis, dtype=FP32)
    wq = np.asarray(wq, dtype=FP32)
    wk = np.asarray(wk, dtype=FP32)
    wv = np.asarray(wv, dtype=FP32)
    wo = np.asarray(wo, dtype=FP32)

    xf = x.reshape(BL, DIM)
    xt = np.ascontiguousarray(
        xf.reshape(NT, 128, KT, 128).transpose(0, 3, 2, 1)
    ).reshape(NT, 128, DIM).astype(BF16)

    cosb = np.tile(freqs_cis[0], (B, 1))  # [BL, 32]
    sinb = np.tile(freqs_cis[1], (B, 1))
    cosq = np.ascontiguousarray((cosb * 0.125).reshape(NT, 128, 32)).astype(FP32)
    sinq = np.ascontiguousarray((sinb * 0.125).reshape(NT, 128, 32)).astype(FP32)
    cosk = np.ascontiguousarray(cosb.reshape(NT, 128, 32)).astype(FP32)
    sink = np.ascontiguousarray(sinb.reshape(NT, 128, 32)).astype(FP32)

    mask = np.triu(np.ones((128, 128), dtype=FP32)).astype(BF16)
    ident = np.eye(128, dtype=FP32).astype(BF16)
    wot = np.ascontiguousarray(wo.T.reshape(KT, 128, DIM)).astype(BF16)

    in_maps = []
    for r in range(R):
        wq_sh = wq[r * CL:(r + 1) * CL]          # [256, 2048]
        wk_sh = wk[r * HD:(r + 1) * HD]          # [64, 2048]
        wv_sh = wv[r * HD:(r + 1) * HD]
        wcomb = np.concatenate([wq_sh.T, wk_sh.T, wv_sh.T], axis=1)  # [2048, 384]
        wcomb = np.ascontiguousarray(wcomb.reshape(KT, 128, 384)).astype(BF16)
        in_maps.append({
            "xt": xt, "wcomb": wcomb,
            "cosq": cosq, "sinq": sinq, "cosk": cosk, "sink": sink,
            "mask": mask, "ident": ident, "wot": wot,
        })
    return in_maps


def run(inputs, trace=False, trace_cores=None):
    nc = _build()
    in_maps = _prep_inputs(**inputs)
    res = run_bass_kernel_spmd(
        nc, in_maps, core_ids=list(range(R)), trace=trace,
        trace_cores=trace_cores,
    )
    shards = [np.asarray(res.results[r]["out"], dtype=FP32) for r in range(R)]
    y = np.concatenate(shards, axis=0).reshape(B, L, DIM)
    return y, res


def kernel(x, freqs_cis, wq, wk, wv, wo):
    y, _ = run(dict(x=x, freqs_cis=freqs_cis, wq=wq, wk=wk, wv=wv, wo=wo))
    return y


# revision 20
# speedup vs baseline: 1.0870x; 1.0870x over previous
"""Distributed Trainium2 kernel for GQA causal attention (B=2, L=2048, DIM=2048,
H=32 q-heads, KV=8 kv-heads, HD=64), tensor-parallel over heads across 8 cores.

Per-core pipeline (core r owns q heads 4r..4r+3 and kv head r):
  P1: qkv = x @ [wqT|wkT|wvT] shard (bf16 matmuls, f32 psum), RoPE on q/k,
      PE-transpose q/k into [head_dim, seq] layout; keep v in [seq, head_dim].
  P2: causal attention per (batch, head) with scores computed transposed
      (S^T[j,i] so softmax probs feed the PV matmul without transposing P),
      exp without max-subtraction (scores are O(5), f32 psum), denominator
      fused as a ones-column of the V stationary, per-512-column chunks.
  P2.5: AllToAll switches head-sharding -> sequence-sharding (2MB bf16).
  P3: full output projection for this core's 512-row sequence shard.
Host: concatenates the 8 row-shards.
"""

import sys

sys.path.insert(0, "/opt/trn_rl_repo")

import numpy as np
import ml_dtypes

from concourse import bass, bacc, mybir, tile
from concourse.bass_utils import run_bass_kernel_spmd

# ---- custom DVE op: Schraudolph exp + parabola mantissa polish ------------
# out = Src1 * ((f^2 - |f|) + 1/k), f = Src0 - rint(Src0) via the magic-add;
# since f^2-|f| = fhat^2-fhat for fhat = frac(Src0), this multiplies the
# Schraudolph value y = 2^x*(1+fhat)/2^fhat by (1 + k*(fhat^2-fhat))/k, a
# two-parameter fit of 2^fhat/(1+fhat) (0.4% rms). The global 1/k scale
# cancels in the softmax ratio; ScalarE pieces match it via an exp bias.
from concourse import dve_ops as _D
from concourse.dve_spec import (AluOp as _AluOp, Bin as _Bin, Spec as _Spec,
                                Src0 as _Src0, Src1 as _Src1, C0 as _C0,
                                C1 as _C1, Zero as _Zero, sq as _sq,
                                lower as _dve_lower)
from concourse.dve_uop import DveOpSpec as _DveOpSpec

MAGIC = 12582912.0
KPAR = 0.2150
INVK = 1.0 / KPAR
ASCALE = 0.996908
EXP_A1 = float(np.float32(2 ** 23))
EXP_B1 = float(np.float32(127 * 2 ** 23) + np.float32(2 ** 23 * np.log2(ASCALE)))
LOG2E = float(np.float64(1.0) / np.log(2.0))
ACT_EXP_SCALE = float(np.log(2.0))
ACT_EXP_BIAS = float(np.log(INVK))


def _polish_ref(in0, in1, s0, s1, imm2):
    f = (in0 - np.rint(in0)).astype(np.float32)
    return ((f * f - np.abs(f)) + np.float32(s1)) * in1


def _install_polish():
    for op in _D.OPS:
        if op.name == "EXP_POLISH_ANT":
            return op
    f = _Bin(_AluOp.SUBTRACT, _Src0,
             _Bin(_AluOp.SUBTRACT, _Bin(_AluOp.ADD, _Src0, _C0), _C0))
    g = _Bin(_AluOp.SUBTRACT, _sq(f), _Bin(_AluOp.ABSOLUTE_DIFF, f, _Zero))
    body = _Bin(_AluOp.MULTIPLY, _Bin(_AluOp.ADD, g, _C1), _Src1)
    row = _D._CUSTOM_DVE_ROW_BASE + len(_D.OPS)
    spec = _Spec(body=body, reference=_polish_ref)
    sha = _DveOpSpec(name="EXP_POLISH_ANT", opcode=row,
                     uops=_dve_lower(spec, ver="v3"), rd1_en=True).sha("v3")
    op = _D.DveOp("EXP_POLISH_ANT", spec, subdim=False,
                  uops_sha={"v3": sha, "v4": sha})
    _D.OPS.append(op)
    _D._SUB_OPCODE_FOR_NAME["EXP_POLISH_ANT"] = row
    _D.CUSTOM_DVE_SPECS["EXP_POLISH_ANT"] = op.spec
    return op


EXP_POLISH = _install_polish()


BF16 = ml_dtypes.bfloat16
FP32 = np.float32

R = 8            # cores
B, L, DIM = 2, 2048, 2048
H, KV, HD = 32, 8, 64
HL = H // R      # 4 local q heads per core
BL = B * L       # 4096
NT = BL // 128   # 32 row tiles
KT = DIM // 128  # 16 contraction tiles
CL = HL * HD     # 256 local q/out dims
NCH = L // 512   # 4 i-chunks per batch
SH = BL // R     # 512 seq rows per core in phase 3

_BF = mybir.dt.bfloat16
_F32 = mybir.dt.float32
_I32 = mybir.dt.int32

_cache = {}


def _emit(nc, t):
    mult = mybir.AluOpType.mult
    add = mybir.AluOpType.add
    sub = mybir.AluOpType.subtract
    Exp = mybir.ActivationFunctionType.Exp
    Copy = mybir.ActivationFunctionType.Copy
    tc = t["tc"]

    with tc.tile_pool(name="persist", bufs=1) as P, \
         tc.tile_pool(name="dramp", bufs=1, space="DRAM") as DP:
        wcomb_sb = P.tile([128, KT * 384], _BF, name="wcomb_sb")
        cosq_sb = P.tile([128, NT * 32], _F32, name="cosq_sb")
        sinq_sb = P.tile([128, NT * 32], _F32, name="sinq_sb")
        cosk_sb = P.tile([128, NT * 32], _F32, name="cosk_sb")
        sink_sb = P.tile([128, NT * 32], _F32, name="sink_sb")
        mask_sb = P.tile([128, 128], _BF, name="mask_sb")
        ident_sb = P.tile([128, 128], _BF, name="ident_sb")
        qT = P.tile([64, HL * BL], _BF, name="qT")
        kT = P.tile([64, BL], _BF, name="kT")
        v1 = P.tile([128, NT * 65], _BF, name="v1")
        out_sb = P.tile([128, NT * CL], _BF, name="out_sb")
        ebias_sb = P.tile([128, 1], _F32, name="ebias_sb")
        escale_sb = P.tile([128, 1], _F32, name="escale_sb")
        wot_sb = P.tile([128, KT * DIM], _BF, name="wot_sb")

        a2a_in = [DP.tile([L, CL], _BF, tag=f"a2a_in{b}", name=f"a2a_in{b}")
                  for b in range(B)]
        a2a_out = [DP.tile([L, CL], _BF, tag=f"a2a_out{b}", name=f"a2a_out{b}")
                   for b in range(B)]

        # --- constant / weight loads (all host-prepacked [128, F]) --------
        for kt in range(KT):
            nc.sync.dma_start(out=wcomb_sb[:, kt * 384:(kt + 1) * 384],
                              in_=t["wcomb"].ap()[:, kt * 384:(kt + 1) * 384])
        for name, sb in (("cosq", cosq_sb), ("sinq", sinq_sb),
                         ("cosk", cosk_sb), ("sink", sink_sb)):
            nc.sync.dma_start(out=sb[:], in_=t[name].ap())
        nc.sync.dma_start(out=mask_sb[:], in_=t["mask"].ap())
        nc.sync.dma_start(out=ident_sb[:], in_=t["ident"].ap())
        nc.vector.memset(
            v1[:].rearrange("p (t c) -> p t c", c=65)[:, :, 64:65], 1.0
        )
        nc.vector.memset(ebias_sb[:], ACT_EXP_BIAS)
        nc.vector.memset(escale_sb[:], ACT_EXP_SCALE)

        # --- phase 1: qkv projection + rope + transposes ------------------
        with tc.tile_pool(name="pp1", bufs=1, space="PSUM") as pp1, \
             tc.tile_pool(name="sp1", bufs=1) as sp1:
            for it in range(NT):
                xt_sb = sp1.tile([128, DIM], _BF, tag="xt", bufs=3, name=f"xt{it}")
                nc.sync.dma_start(out=xt_sb[:], in_=t["xt"].ap()[it])
                qkv = pp1.tile([128, 384], _F32, tag="qkv", bufs=2, name=f"qkv{it}")
                for kt in range(KT):
                    nc.tensor.matmul(
                        qkv[:],
                        xt_sb[:, kt * 128:(kt + 1) * 128],
                        wcomb_sb[:, kt * 384:(kt + 1) * 384],
                        start=(kt == 0), stop=(kt == KT - 1),
                    )
                # rope on q (4 heads, pairs interleaved along free dim)
                qrot = sp1.tile([128, CL], _BF, tag="qrot", bufs=2, name=f"qrot{it}")
                q3 = qkv[:, 0:CL].rearrange("p (h c) -> p h c", h=HL)
                qe, qo = q3[:, :, 0:64:2], q3[:, :, 1:64:2]
                cq = cosq_sb[:, it * 32:(it + 1) * 32].unsqueeze(1).broadcast_to([128, HL, 32])
                sq = sinq_sb[:, it * 32:(it + 1) * 32].unsqueeze(1).broadcast_to([128, HL, 32])
                t1 = sp1.tile([128, 128], _F32, tag="t1", bufs=2, name=f"t1_{it}")
                t2 = sp1.tile([128, 128], _F32, tag="t2", bufs=2, name=f"t2_{it}")
                t3 = sp1.tile([128, 128], _F32, tag="t3", bufs=2, name=f"t3_{it}")
                t4 = sp1.tile([128, 128], _F32, tag="t4", bufs=2, name=f"t4_{it}")
                v1_ = lambda x: x[:].rearrange("p (h c) -> p h c", h=HL)
                qr3 = qrot[:].rearrange("p (h c) -> p h c", h=HL)
                nc.vector.tensor_tensor(v1_(t1), qe, cq, mult)
                nc.vector.tensor_tensor(v1_(t2), qo, sq, mult)
                nc.vector.tensor_tensor(v1_(t3), qe, sq, mult)
                nc.vector.tensor_tensor(v1_(t4), qo, cq, mult)
                nc.vector.tensor_tensor(qr3[:, :, 0:64:2], v1_(t1), v1_(t2), sub)
                nc.vector.tensor_tensor(qr3[:, :, 1:64:2], v1_(t3), v1_(t4), add)
                # rope on k (single kv head)
                krot = sp1.tile([128, 64], _BF, tag="krot", bufs=2, name=f"krot{it}")
                kbase = qkv[:, CL:CL + 64]
                ke, ko = kbase[:, 0:64:2], kbase[:, 1:64:2]
                ck = cosk_sb[:, it * 32:(it + 1) * 32]
                sk = sink_sb[:, it * 32:(it + 1) * 32]
                k1 = sp1.tile([128, 32], _F32, tag="k1", bufs=2, name=f"k1_{it}")
                k2 = sp1.tile([128, 32], _F32, tag="k2", bufs=2, name=f"k2_{it}")
                k3 = sp1.tile([128, 32], _F32, tag="k3", bufs=2, name=f"k3_{it}")
                k4 = sp1.tile([128, 32], _F32, tag="k4", bufs=2, name=f"k4_{it}")
                nc.vector.tensor_tensor(k1[:], ke, ck, mult)
                nc.vector.tensor_tensor(k2[:], ko, sk, mult)
                nc.vector.tensor_tensor(k3[:], ke, sk, mult)
                nc.vector.tensor_tensor(k4[:], ko, ck, mult)
                nc.vector.tensor_tensor(krot[:, 0:64:2], k1[:], k2[:], sub)
                nc.vector.tensor_tensor(krot[:, 1:64:2], k3[:], k4[:], add)
                # v copy (f32 psum -> bf16 sbuf) — small, ok on ScalarE
                nc.scalar.activation(v1[:, it * 65:it * 65 + 64], qkv[:, 320:384], Copy)
                # transposes: q -> qT, k -> kT
                for h in range(HL):
                    trq = pp1.tile([64, 128], _BF, tag="trq", bufs=2,
                                   name=f"trq{it}_{h}")
                    nc.tensor.transpose(trq[:], qrot[:, h * 64:(h + 1) * 64],
                                        ident_sb[:])
                    nc.scalar.activation(
                        qT[:, h * BL + it * 128: h * BL + (it + 1) * 128],
                        trq[:], Copy)
                trk = pp1.tile([64, 128], _BF, tag="trk", bufs=2, name=f"trk{it}")
                nc.tensor.transpose(trk[:], krot[:], ident_sb[:])
                nc.scalar.activation(kT[:, it * 128:(it + 1) * 128], trk[:], Copy)

        # wot loads (needed in phase 3; emitted here so DMA happens in the
        # background during phase 2)
        for q in range(4):
            w = KT * DIM // 4
            nc.sync.dma_start(
                out=wot_sb[:, q * w:(q + 1) * w],
                in_=t["wot"].ap()[:, q * w:(q + 1) * w])

        # --- phases 2+3: attention, per-batch all-to-all, out projection ---
        # PSUM budget (8 banks): S x4, o x2, ot x1, y x1.
        with tc.tile_pool(name="ppS", bufs=1, space="PSUM") as ppS, \
             tc.tile_pool(name="ppO", bufs=1, space="PSUM") as ppO, \
             tc.tile_pool(name="ppT", bufs=1, space="PSUM") as ppT, \
             tc.tile_pool(name="ppY", bufs=1, space="PSUM") as ppY, \
             tc.tile_pool(name="sp2", bufs=1) as sp2, \
             tc.tile_pool(name="sp3", bufs=1) as sp3:

            def p3_half(b):
                """Project this batch's 256 seq rows (after its all-to-all)."""
                lhs = []
                for ct in range(KT):
                    lt = sp3.tile([128, 256], _BF, tag=f"lhs{b}_{ct}",
                                  name=f"lhs{b}_{ct}")
                    s, ch = ct // 2, ct % 2
                    nc.sync.dma_start(
                        out=lt[:],
                        in_=a2a_out[b][256 * s:256 * (s + 1),
                                       ch * 128:(ch + 1) * 128],
                        transpose=True)
                    lhs.append(lt)
                for it2 in range(2):
                    for nck in range(4):
                        y_ps = ppY.tile([128, 512], _F32, tag="y", bufs=1,
                                        name=f"y{b}_{it2}_{nck}")
                        for ct in range(KT):
                            nc.tensor.matmul(
                                y_ps[:],
                                lhs[ct][:, it2 * 128:(it2 + 1) * 128],
                                wot_sb[:, ct * DIM + nck * 512:
                                       ct * DIM + nck * 512 + 512],
                                start=(ct == 0), stop=(ct == KT - 1))
                        y_sb = sp3.tile([128, 512], _F32, tag="ysb", bufs=2,
                                        name=f"ysb{b}_{it2}_{nck}")
                        nc.scalar.activation(y_sb[:], y_ps[:], Copy)
                        nc.sync.dma_start(
                            out=t["out"][b * 256 + it2 * 128:
                                         b * 256 + (it2 + 1) * 128,
                                         nck * 512:(nck + 1) * 512],
                            in_=y_sb[:])

            t["ebias"], t["escale"] = ebias_sb, escale_sb
            exp_ctr = [0]

            def emit_exp(p_bf, p_int, s_ps, a):
                # Scores arrive base-2 (q pre-scaled by log2e/8 on the host).
                # 1/3 of pieces: exact exp on ScalarE (kept under ~25% duty or
                # the PE clock gets power-clamped), scale-matched via bias.
                # 2/3: Schraudolph int32 exp + one custom-DVE parabola polish
                # (0.4% scatter); its global 1/k scale cancels in the softmax.
                exp_ctr[0] += 1
                if exp_ctr[0] % 2 == 0:
                    nc.scalar.activation(p_bf[:, a:512], s_ps[:, a:512], Exp,
                                         bias=t["ebias"][:], scale=t["escale"][:])
                else:
                    nc.vector.tensor_scalar(
                        p_int[:, a:512], s_ps[:, a:512], EXP_A1, EXP_B1,
                        mult, add)
                    nc.vector._custom_dve(
                        EXP_POLISH, out=p_bf[:, a:512], in0=s_ps[:, a:512],
                        in1=p_int[:, a:512].bitcast(_F32),
                        s0=MAGIC, s1=INVK)

            for b in range(B):
                for h in range(HL):
                    for c in range(NCH):
                        i0 = c * 512
                        o_ps = ppO.tile([65, 512], _F32, tag="o", bufs=2,
                                        name=f"o{b}_{h}_{c}")
                        njt = 4 * c + 4
                        for jt in range(njt):
                            j0 = jt * 128
                            a = max(0, j0 - i0)
                            n = 512 - a
                            s_ps = ppS.tile([128, 512], _F32, tag="S", bufs=4,
                                            name=f"s{b}_{h}_{c}_{jt}")
                            qcol = h * BL + b * L + i0 + a
                            nc.tensor.matmul(
                                s_ps[:, a:512],
                                kT[0:64, b * L + j0: b * L + j0 + 128],
                                qT[0:64, qcol:qcol + n],
                                start=True, stop=True,
                            )
                            p_int = sp2.tile([128, 512], _I32, tag="PI", bufs=4,
                                             name=f"pi{b}_{h}_{c}_{jt}")
                            p_bf = sp2.tile([128, 512], _BF, tag="P", bufs=6,
                                            name=f"p{b}_{h}_{c}_{jt}")
                            emit_exp(p_bf, p_int, s_ps, a)
                            if jt >= 4 * c:
                                nc.gpsimd.tensor_tensor(
                                    p_bf[:, a:a + 128], p_bf[:, a:a + 128],
                                    mask_sb[:], mult)
                            nc.tensor.matmul(
                                o_ps[0:65, a:512],
                                v1[:, (b * 16 + jt) * 65:(b * 16 + jt) * 65 + 65],
                                p_bf[:, a:512],
                                start=(jt == 0), stop=(jt == njt - 1),
                                skip_group_check=True,
                            )
                        # normalize: one fused transpose-group per chunk
                        o_sb = sp2.tile([65, 512], _BF, tag="osb", bufs=2,
                                        name=f"osb{b}_{h}_{c}")
                        nc.scalar.activation(o_sb[:], o_ps[0:65, :], Copy)
                        ot4 = ppT.tile([128, 264], _BF, tag="ot", bufs=1,
                                       name=f"ot{b}_{h}_{c}")
                        for q4 in range(4):
                            nc.tensor.transpose(
                                ot4[:, q4 * 66:q4 * 66 + 65],
                                o_sb[:, q4 * 128:(q4 + 1) * 128],
                                ident_sb[0:65, 0:65])
                        rcp4 = sp2.tile([128, 4], _F32, tag="rcp", bufs=2,
                                        name=f"rcp{b}_{h}_{c}")
                        ot4v = ot4[:].rearrange("p (q c) -> p q c", q=4)
                        nc.vector.reciprocal(rcp4[:], ot4v[:, :, 64:65])
                        git0 = b * 16 + c * 4
                        dst = out_sb[:, git0 * CL:(git0 + 4) * CL] \
                            .rearrange("p (q c) -> p q c", c=CL)[:, :, h * 64:
                                                                 h * 64 + 64]
                        nc.vector.tensor_tensor(
                            dst, ot4v[:, :, 0:64],
                            rcp4[:].unsqueeze(2).broadcast_to([128, 4, 64]),
                            mult)
                    # interleave the previous batch's projection into b=1's
                    # emission so its priority sits between attention work
                    if b == 1 and h == 1:
                        p3_half(0)
                # stage this batch's rows for its all-to-all
                for lt_ in range(16):
                    git = b * 16 + lt_
                    nc.sync.dma_start(
                        out=a2a_in[b][lt_ * 128:(lt_ + 1) * 128, :],
                        in_=out_sb[:, git * CL:(git + 1) * CL])
                nc.gpsimd.collective_compute(
                    "AllToAll", mybir.AluOpType.bypass,
                    replica_groups=[list(range(R))],
                    ins=[a2a_in[b][:].opt()],
                    outs=[a2a_out[b][:].opt()],
                )
            p3_half(1)


def _build():
    if "nc" in _cache:
        return _cache["nc"]
    nc = bacc.Bacc("TRN2", target_bir_lowering=False, debug=False,
                   enable_asserts=False, num_devices=R)
    t = {}
    t["xt"] = nc.dram_tensor("xt", [NT, 128, DIM], _BF, kind="ExternalInput")
    t["wcomb"] = nc.dram_tensor("wcomb", [128, KT * 384], _BF, kind="ExternalInput")
    for name in ("cosq", "sinq", "cosk", "sink"):
        t[name] = nc.dram_tensor(name, [128, NT * 32], _F32, kind="ExternalInput")
    t["mask"] = nc.dram_tensor("mask", [128, 128], _BF, kind="ExternalInput")
    t["ident"] = nc.dram_tensor("ident", [128, 128], _BF, kind="ExternalInput")
    t["wot"] = nc.dram_tensor("wot", [128, KT * DIM], _BF, kind="ExternalInput")
    t["out"] = nc.dram_tensor("out", [SH, DIM], _F32, kind="ExternalOutput")

    with tile.TileContext(nc) as tc:
        t["tc"] = tc
        _emit(nc, t)
    nc.compile()
    _cache["nc"] = nc
    return nc


def _prep_inputs(x, freqs_cis, wq, wk, wv, wo):
    x = np.asarray(x, dtype=FP32)
    freqs_cis = np.asarray(freqs_cis, dtype=FP32)
    wq = np.asarray(wq, dtype=FP32)
    wk = np.asarray(wk, dtype=FP32)
    wv = np.asarray(wv, dtype=FP32)
    wo = np.asarray(wo, dtype=FP32)

    xf = x.reshape(BL, DIM)
    xt = np.ascontiguousarray(
        xf.reshape(NT, 128, KT, 128).transpose(0, 3, 2, 1)
    ).reshape(NT, 128, DIM).astype(BF16)

    def pack128(a3):
        # [NT, 128, C] -> [128, NT*C] with row p = concat over tiles
        n, _, c = a3.shape
        return np.ascontiguousarray(a3.transpose(1, 0, 2).reshape(128, n * c))

    cosb = np.tile(freqs_cis[0], (B, 1))  # [BL, 32]
    sinb = np.tile(freqs_cis[1], (B, 1))
    qs = 0.125 * LOG2E
    cosq = pack128((cosb * qs).reshape(NT, 128, 32)).astype(FP32)
    sinq = pack128((sinb * qs).reshape(NT, 128, 32)).astype(FP32)
    cosk = pack128(cosb.reshape(NT, 128, 32)).astype(FP32)
    sink = pack128(sinb.reshape(NT, 128, 32)).astype(FP32)

    mask = np.triu(np.ones((128, 128), dtype=FP32)).astype(BF16)
    ident = np.eye(128, dtype=FP32).astype(BF16)
    wot = pack128(wo.T.reshape(KT, 128, DIM)).astype(BF16)

    in_maps = []
    for r in range(R):
        wq_sh = wq[r * CL:(r + 1) * CL]          # [256, 2048]
        wk_sh = wk[r * HD:(r + 1) * HD]          # [64, 2048]
        wv_sh = wv[r * HD:(r + 1) * HD]
        wcomb = np.concatenate([wq_sh.T, wk_sh.T, wv_sh.T], axis=1)  # [2048, 384]
        wcomb = pack128(wcomb.reshape(KT, 128, 384)).astype(BF16)
        in_maps.append({
            "xt": xt, "wcomb": wcomb,
            "cosq": cosq, "sinq": sinq, "cosk": cosk, "sink": sink,
            "mask": mask, "ident": ident, "wot": wot,
        })
    return in_maps


def run(inputs, trace=False, trace_cores=None):
    nc = _build()
    in_maps = _prep_inputs(**inputs)
    res = run_bass_kernel_spmd(
        nc, in_maps, core_ids=list(range(R)), trace=trace,
        trace_cores=trace_cores,
    )
    shards = [np.asarray(res.results[r]["out"], dtype=FP32) for r in range(R)]
    y = np.empty((BL, DIM), dtype=FP32)
    for r in range(R):
        y[256 * r:256 * (r + 1)] = shards[r][0:256]
        y[L + 256 * r:L + 256 * (r + 1)] = shards[r][256:512]
    return y.reshape(B, L, DIM), res


def kernel(x, freqs_cis, wq, wk, wv, wo):
    y, _ = run(dict(x=x, freqs_cis=freqs_cis, wq=wq, wk=wk, wv=wv, wo=wo))
    return y
